# revision 7
# baseline (speedup 1.0000x reference)
"""BinLinear TRN2 kernel: out = x @ sign(weight).T + sign(bias).

Full shapes: x [8192, 4096] f32, weight [4096, 4096] f32, bias [4096] f32
-> out [8192, 4096] f32.

Sharding (8 NeuronCores): 2D grid, 4-way over tokens x 2-way over output
features. Each core computes out_c = x_c @ sign(w_c).T + sign(b_c) with
x_c [2048, 4096], w_c [2048, 4096], b_c [2048] -> out_c [2048, 2048].
The host only slices inputs and stitches the 4x2 output grid back together.

Per-core device program (fp16 single-pass matmul, everything on device):
  - One SWDGE cast chain streams 128-row slabs of w and x fp32->fp16
    DRAM->SBUF, interleaved x0,w0,w1,x1,w2,w3,... so the PE's available
    work frontier (arrived w-pairs x arrived x-slabs) grows quadratically
    while the stream is linear.
  - Each staged slab is XBAR dma-transposed SBUF->SBUF into resident
    wT pair-tiles [128, 32kt, 256feat] (8 of them) / a ring of xT slab
    tiles [128, 32kt, 128tok].
  - Weight slabs are binarized in place by one fused DVE op:
    (w16 > 0) - 0.5 = 0.5*sign(w) (exact-zero weights use a 3-op exact
    variant selected on the host; never needed for randn inputs).
  - PE: uniform [128-token, 256-feature] PSUM cells: one K=1 matmul
    seeds the bias row (0.5-ones^T @ sign(b)), then 32 K=128 fp16
    matmuls.  N=256 runs at full PE rate (107ns matmul still hides the
    97ns LDWEIGHTS) and lets a cell depend on a single w pair-tile, so
    compute starts ~25us in on partially-arrived weights.
  - Strict engine separation so no in-order queue mixes early-ready and
    late-ready work: Pool=casts, SP=XBARs, DVE=weight signs,
    Act=psum*2 copy-back + output DMA, PE=matmuls.
"""

import sys

if "/opt/trn_rl_repo" not in sys.path:
    sys.path.insert(0, "/opt/trn_rl_repo")

from contextlib import ExitStack

import numpy as np

import concourse.bass as bass
import concourse.mybir as mybir
import concourse.tile as tile
from concourse import bacc
from concourse.bass_utils import run_bass_kernel_spmd
from concourse.tile_rust import add_dep_helper

N_TOK, D_IN, D_OUT = 8192, 4096, 4096
TOK_WAYS, OUT_WAYS = 4, 2
N_CORES = TOK_WAYS * OUT_WAYS
TOK_SH = N_TOK // TOK_WAYS    # 2048 tokens per core
OUT_SH = D_OUT // OUT_WAYS    # 2048 out features per core

P = 128
KT = D_IN // P                # 32 contraction subtiles
NFREE = 256                   # PSUM free dim per matmul (one w pair-tile)
NSL = TOK_SH // P             # 16 token slabs
NWS = OUT_SH // P             # 16 weight slabs
NPAIR = NWS // 2              # 8 weight pair-tiles

F16 = mybir.dt.float16
F32 = mybir.dt.float32


def _build(exact_sign: bool):
    """Build the per-core SPMD program."""
    nc = bacc.Bacc("TRN2", target_bir_lowering=False, debug=False,
                   num_devices=N_CORES)
    x = nc.dram_tensor("x", [TOK_SH, D_IN], F32, kind="ExternalInput")
    w = nc.dram_tensor("w", [OUT_SH, D_IN], F32, kind="ExternalInput")
    b = nc.dram_tensor("b", [1, OUT_SH], F32, kind="ExternalInput")
    out = nc.dram_tensor("out", [TOK_SH, OUT_SH], F32, kind="ExternalOutput")

    PRO_S = 4 if exact_sign else 5   # x slabs resident during the prologue
    RING = PRO_S                     # xT ring size

    with ExitStack() as ctx:
        tc = ctx.enter_context(tile.TileContext(nc))
        wTp = ctx.enter_context(tc.tile_pool(name="wTp", bufs=NPAIR))
        xTp = ctx.enter_context(tc.tile_pool(name="xTp", bufs=RING))
        stagep = ctx.enter_context(tc.tile_pool(name="stagep", bufs=2))
        sgtmp = ctx.enter_context(tc.tile_pool(name="sgtmp", bufs=2))
        osbp = ctx.enter_context(tc.tile_pool(name="osbp", bufs=3))
        constp = ctx.enter_context(tc.tile_pool(name="constp", bufs=1))
        mmps = ctx.enter_context(tc.tile_pool(name="mmps", bufs=8, space="PSUM"))

        def sign_half_inplace(ap, tmp_shape, tag):
            """ap = 0.5*sign(ap) elementwise on fp16 data (in place), DVE."""
            if exact_sign:
                t1 = sgtmp.tile(tmp_shape, F16, tag=tag, name=f"{tag}_t")
                nc.vector.tensor_scalar(t1[:], ap, 0.0, None, mybir.AluOpType.is_lt)
                nc.vector.tensor_scalar(ap, ap, 0.0, None, mybir.AluOpType.is_gt)
                nc.vector.tensor_tensor(ap, ap, t1[:], mybir.AluOpType.subtract)
                nc.vector.tensor_scalar(ap, ap, 0.5, None, mybir.AluOpType.mult)
            else:
                nc.vector.tensor_scalar(
                    ap, ap, 0.0, 0.5,
                    mybir.AluOpType.is_gt, mybir.AluOpType.subtract,
                )

        # ---- SWDGE cast chain (DRAM fp32 -> SBUF fp16), nosync-ordered so
        # slabs complete in stream order without hard pacing stalls.
        last_swdge = [None]

        def swdge_cast(dst_ap, src_ap):
            inst = nc.gpsimd.dma_start(dst_ap, src_ap)
            if last_swdge[0] is not None:
                add_dep_helper(inst.ins, last_swdge[0].ins, sync=False,
                               reason="SWDGE cast order")
            last_swdge[0] = inst
            return inst

        # ---- constants + bias row: brow = sign(b) (+-1, exact) on Act;
        # the K=1 bias seed matmul uses 0.5-valued ones so psum gets
        # 0.5*sign(b), matching the 0.5*sign(w) accumulation, x2 on copy.
        ones = constp.tile([1, P], F16)
        nc.gpsimd.memset(ones[:], 0.5)
        bf32 = constp.tile([1, OUT_SH], F32)
        nc.scalar.dma_start(bf32[:], b[:])
        brow = constp.tile([1, OUT_SH], F16)
        nc.scalar.activation(brow[:], bf32[:],
                             mybir.ActivationFunctionType.Sign)

        # ---- resident weight pair-tiles and the xT ring
        wT = [wTp.tile([P, KT, NFREE], F16, tag="wT", name=f"wT{q}")
              for q in range(NPAIR)]
        xT = [None] * NSL

        def w_slab(j):
            """Stream weight slab j: cast, transpose, binarize."""
            st = stagep.tile([P, D_IN], F16, tag="stage", name=f"wst{j}")
            swdge_cast(st[:], w[j * P:(j + 1) * P, :])
            q, jj = j // 2, j % 2
            dst = wT[q][:, :, jj * P:(jj + 1) * P]
            nc.sync.dma_start_transpose(dst, st[:])
            sign_half_inplace(dst, [P, KT, P], "wsg")

        def x_slab(s):
            """Stream token slab s: cast, transpose into ring slot s%RING."""
            st = stagep.tile([P, D_IN], F16, tag="stage", name=f"xst{s}")
            swdge_cast(st[:], x[s * P:(s + 1) * P, :])
            xT[s] = xTp.tile([P, KT, P], F16, tag="xT", name=f"xT{s}")
            nc.sync.dma_start_transpose(xT[s][:], st[:])

        def cell(q, s):
            """One [128-token, 256-feature] output cell: bias seed + 32
            matmuls, Act-engine x2 copy-back, DMA out."""
            psum = mmps.tile([P, NFREE], F32, tag="mm", name="psum")
            nc.tensor.matmul(
                psum[:], ones[:], brow[0:1, q * NFREE:(q + 1) * NFREE],
                start=True, stop=False,
            )
            for kt in range(KT):
                nc.tensor.matmul(
                    psum[:], xT[s][:, kt, :], wT[q][:, kt, :],
                    start=False, stop=(kt == KT - 1),
                )
            osb = osbp.tile([P, NFREE], F32, tag="osb", name="osb")
            nc.scalar.activation(osb[:], psum[:],
                                 mybir.ActivationFunctionType.Copy, 0.0, 2.0)
            nc.scalar.dma_start(
                out[s * P:(s + 1) * P, q * NFREE:(q + 1) * NFREE], osb[:])

        # ---- stream + prologue wavefront over (pair r, slab s) cells.
        # Stream order: x0,w0,w1 | x1,w2,w3 | ... | rest of w pairs |
        # x{PRO_S}..x15.  Cell (q, s) is emitted at round max(q, s) so it
        # only needs arrived data.
        x_slab(0)
        for r in range(NPAIR):
            w_slab(2 * r)
            w_slab(2 * r + 1)
            if r + 1 < PRO_S:
                x_slab(r + 1)
            for s in range(min(r + 1, PRO_S)):
                cell(r, s)
            if r < PRO_S - 1:
                for q in range(r + 1):
                    cell(q, r + 1)

        # ---- bulk: remaining token slabs, slab-major (wT fully resident).
        for s in range(PRO_S, NSL):
            x_slab(s)
            for q in range(NPAIR):
                cell(q, s)

    nc.finalize()
    return nc


_cache = {}


def _get_nc(exact_sign: bool):
    if exact_sign not in _cache:
        _cache[exact_sign] = _build(exact_sign)
    return _cache[exact_sign]


def kernel(x: np.ndarray, weight: np.ndarray, bias: np.ndarray) -> np.ndarray:
    x = np.ascontiguousarray(np.asarray(x, dtype=np.float32))
    weight = np.ascontiguousarray(np.asarray(weight, dtype=np.float32))
    bias = np.ascontiguousarray(np.asarray(bias, dtype=np.float32))
    assert x.shape == (N_TOK, D_IN) and weight.shape == (D_OUT, D_IN)

    # (w > 0) - 0.5 equals 0.5*sign(w) only when no exact zeros exist;
    # fall back to the exact 3-op sign variant otherwise (bias zeros are
    # handled exactly by the Act-engine Sign either way).
    exact_sign = bool((weight == 0.0).any())
    nc = _get_nc(exact_sign)

    in_maps = []
    for tg in range(TOK_WAYS):
        for og in range(OUT_WAYS):
            in_maps.append({
                "x": np.ascontiguousarray(x[tg * TOK_SH:(tg + 1) * TOK_SH, :]),
                "w": np.ascontiguousarray(weight[og * OUT_SH:(og + 1) * OUT_SH, :]),
                "b": np.ascontiguousarray(
                    bias[og * OUT_SH:(og + 1) * OUT_SH].reshape(1, OUT_SH)),
            })

    res = run_bass_kernel_spmd(nc, in_maps, list(range(N_CORES)))

    out = np.empty((N_TOK, D_OUT), dtype=np.float32)
    c = 0
    for tg in range(TOK_WAYS):
        for og in range(OUT_WAYS):
            out[tg * TOK_SH:(tg + 1) * TOK_SH, og * OUT_SH:(og + 1) * OUT_SH] = \
                res.results[c]["out"]
            c += 1
    return out


# revision 8
# speedup vs baseline: 1.0610x; 1.0610x over previous
"""BinLinear TRN2 kernel: out = x @ sign(weight).T + sign(bias).

Full shapes: x [8192, 4096] f32, weight [4096, 4096] f32, bias [4096] f32
-> out [8192, 4096] f32.

Sharding (8 NeuronCores): 2D grid, 4-way over tokens x 2-way over output
features. Each core computes out_c = x_c @ sign(w_c).T + sign(b_c) with
x_c [2048, 4096], w_c [2048, 4096], b_c [2048] -> out_c [2048, 2048].
The host only slices inputs and stitches the 4x2 output grid back together.

Per-core device program (fp16 single-pass matmul, everything on device):
  - One SWDGE cast chain streams 256-row (two-slab) chunks of w and x
    fp32->fp16 DRAM->SBUF (4.2MB ops amortize the per-op DGE overhead),
    interleaved X01,W0,W1,X23,W2..W7,X45.. so the PE's available work
    frontier grows while the stream is linear.  Two pair-stage buffers
    give 4 slabs of cast lookahead over the XBAR consumers.
  - Each staged slab is XBAR dma-transposed SBUF->SBUF into resident
    wT pair-tiles [128, 32kt, 256feat] (8 of them) / a ring of xT slab
    tiles [128, 32kt, 128tok].
  - Weight slabs are binarized in place by one fused DVE op:
    (w16 > 0) - 0.5 = 0.5*sign(w) (exact-zero weights use a 3-op exact
    variant selected on the host; never needed for randn inputs).
  - PE: uniform [128-token, 256-feature] PSUM cells: one K=1 matmul
    seeds the bias row (0.5-ones^T @ sign(b)), then 32 K=128 fp16
    matmuls.  N=256 runs at full PE rate (107ns matmul still hides the
    97ns LDWEIGHTS) and lets a cell depend on a single w pair-tile, so
    compute starts ~25us in on partially-arrived weights.
  - Strict engine separation so no in-order queue mixes early-ready and
    late-ready work: Pool=casts, SP=XBARs, DVE=weight signs,
    Act=bias sign + psum*2 copy-back + output DMA, PE=matmuls.
"""

import sys

if "/opt/trn_rl_repo" not in sys.path:
    sys.path.insert(0, "/opt/trn_rl_repo")

from contextlib import ExitStack

import numpy as np

import concourse.bass as bass
import concourse.mybir as mybir
import concourse.tile as tile
from concourse import bacc
from concourse.bass_utils import run_bass_kernel_spmd
from concourse.tile_rust import add_dep_helper

N_TOK, D_IN, D_OUT = 8192, 4096, 4096
TOK_WAYS, OUT_WAYS = 4, 2
N_CORES = TOK_WAYS * OUT_WAYS
TOK_SH = N_TOK // TOK_WAYS    # 2048 tokens per core
OUT_SH = D_OUT // OUT_WAYS    # 2048 out features per core

P = 128
KT = D_IN // P                # 32 contraction subtiles
NFREE = 256                   # PSUM free dim per matmul (one w pair-tile)
NSL = TOK_SH // P             # 16 token slabs
NWS = OUT_SH // P             # 16 weight slabs
NPAIR = NWS // 2              # 8 weight pair-tiles

F16 = mybir.dt.float16
F32 = mybir.dt.float32


def _build(exact_sign: bool):
    """Build the per-core SPMD program."""
    nc = bacc.Bacc("TRN2", target_bir_lowering=False, debug=False,
                   num_devices=N_CORES)
    x = nc.dram_tensor("x", [TOK_SH, D_IN], F32, kind="ExternalInput")
    w = nc.dram_tensor("w", [OUT_SH, D_IN], F32, kind="ExternalInput")
    b = nc.dram_tensor("b", [1, OUT_SH], F32, kind="ExternalInput")
    out = nc.dram_tensor("out", [TOK_SH, OUT_SH], F32, kind="ExternalOutput")

    # x slabs resident during the prologue / xT ring size.  The exact-sign
    # variant needs DVE temp tiles, so it runs a smaller (slower) config;
    # it is never selected for randn inputs.
    PRO_S = 2 if exact_sign else 4
    RING = PRO_S
    STAGE_BUFS = 1 if exact_sign else 2

    with ExitStack() as ctx:
        tc = ctx.enter_context(tile.TileContext(nc))
        wTp = ctx.enter_context(tc.tile_pool(name="wTp", bufs=NPAIR))
        xTp = ctx.enter_context(tc.tile_pool(name="xTp", bufs=RING))
        stagep = ctx.enter_context(tc.tile_pool(name="stagep", bufs=STAGE_BUFS))
        sgtmp = ctx.enter_context(tc.tile_pool(name="sgtmp", bufs=2))
        osbp = ctx.enter_context(tc.tile_pool(name="osbp", bufs=3))
        constp = ctx.enter_context(tc.tile_pool(name="constp", bufs=1))
        mmps = ctx.enter_context(tc.tile_pool(name="mmps", bufs=8, space="PSUM"))

        def sign_half_inplace(ap, tmp_shape, tag):
            """ap = 0.5*sign(ap) elementwise on fp16 data (in place), DVE."""
            if exact_sign:
                t1 = sgtmp.tile(tmp_shape, F16, tag=tag, name=f"{tag}_t")
                nc.vector.tensor_scalar(t1[:], ap, 0.0, None, mybir.AluOpType.is_lt)
                nc.vector.tensor_scalar(ap, ap, 0.0, None, mybir.AluOpType.is_gt)
                nc.vector.tensor_tensor(ap, ap, t1[:], mybir.AluOpType.subtract)
                nc.vector.tensor_scalar(ap, ap, 0.5, None, mybir.AluOpType.mult)
            else:
                nc.vector.tensor_scalar(
                    ap, ap, 0.0, 0.5,
                    mybir.AluOpType.is_gt, mybir.AluOpType.subtract,
                )

        # ---- SWDGE cast chain (DRAM fp32 -> SBUF fp16), nosync-ordered so
        # chunks complete in stream order without hard pacing stalls.
        last_swdge = [None]

        def swdge_cast(dst_ap, src_ap):
            inst = nc.gpsimd.dma_start(dst_ap, src_ap)
            if last_swdge[0] is not None:
                add_dep_helper(inst.ins, last_swdge[0].ins, sync=False,
                               reason="SWDGE cast order")
            last_swdge[0] = inst
            return inst

        # ---- constants + bias row: brow = sign(b) (+-1, exact) on Act;
        # the K=1 bias seed matmul uses 0.5-valued ones so psum gets
        # 0.5*sign(b), matching the 0.5*sign(w) accumulation, x2 on copy.
        ones = constp.tile([1, P], F16)
        nc.gpsimd.memset(ones[:], 0.5)
        b16 = constp.tile([1, OUT_SH], F16)
        swdge_cast(b16[:], b[:])
        brow = constp.tile([1, OUT_SH], F16)
        nc.scalar.activation(brow[:], b16[:],
                             mybir.ActivationFunctionType.Sign)

        # ---- resident weight pair-tiles and the xT ring
        wT = [wTp.tile([P, KT, NFREE], F16, tag="wT", name=f"wT{q}")
              for q in range(NPAIR)]
        xT = [None] * NSL

        def cast_chunk(src, j0, name):
            """Cast rows [j0*P, (j0+2)*P) of src into a 2-slab stage tile."""
            st = stagep.tile([P, 2, D_IN], F16, tag="stage", name=name)
            swdge_cast(st[:], src[j0 * P:(j0 + 2) * P, :]
                       .rearrange("(a p) d -> p a d", p=P))
            return st

        def w_pair(q):
            """Stream weight pair q (slabs 2q, 2q+1): cast, 2x transpose +
            binarize."""
            st = cast_chunk(w, 2 * q, f"wst{q}")
            for a in range(2):
                dst = wT[q][:, :, a * P:(a + 1) * P]
                nc.sync.dma_start_transpose(dst, st[:, a, :])
                sign_half_inplace(dst, [P, KT, P], "wsg")

        def x_pair(s0):
            """Stream token slabs s0, s0+1: cast, 2x transpose into the
            ring."""
            st = cast_chunk(x, s0, f"xst{s0}")
            for a in range(2):
                s = s0 + a
                xT[s] = xTp.tile([P, KT, P], F16, tag="xT", name=f"xT{s}")
                nc.sync.dma_start_transpose(xT[s][:], st[:, a, :])

        def cell(q, s):
            """One [128-token, 256-feature] output cell: bias seed + 32
            matmuls, Act-engine x2 copy-back, DMA out."""
            psum = mmps.tile([P, NFREE], F32, tag="mm", name="psum")
            nc.tensor.matmul(
                psum[:], ones[:], brow[0:1, q * NFREE:(q + 1) * NFREE],
                start=True, stop=False,
            )
            for kt in range(KT):
                nc.tensor.matmul(
                    psum[:], xT[s][:, kt, :], wT[q][:, kt, :],
                    start=False, stop=(kt == KT - 1),
                )
            osb = osbp.tile([P, NFREE], F32, tag="osb", name="osb")
            nc.scalar.activation(osb[:], psum[:],
                                 mybir.ActivationFunctionType.Copy, 0.0, 2.0)
            nc.scalar.dma_start(
                out[s * P:(s + 1) * P, q * NFREE:(q + 1) * NFREE], osb[:])

        # ---- prologue: stream X01, W0, W1, X23, W2..W7 with the cell
        # wavefront emitted as soon as both sides of a (pair, slab) cell
        # are in flight.
        x_pair(0)
        w_pair(0)
        w_pair(1)
        for q in (0, 1):
            for s in range(min(2, PRO_S)):
                cell(q, s)
        if PRO_S >= 4:
            x_pair(2)
            for q in (0, 1):
                for s in (2, 3):
                    cell(q, s)
        for q in range(2, NPAIR):
            w_pair(q)
            for s in range(PRO_S):
                cell(q, s)

        # ---- bulk: remaining token slabs, slab-major (wT fully resident).
        for s0 in range(PRO_S, NSL, 2):
            x_pair(s0)
            for s in (s0, s0 + 1):
                for q in range(NPAIR):
                    cell(q, s)

    nc.finalize()
    return nc


_cache = {}


def _get_nc(exact_sign: bool):
    if exact_sign not in _cache:
        _cache[exact_sign] = _build(exact_sign)
    return _cache[exact_sign]


def kernel(x: np.ndarray, weight: np.ndarray, bias: np.ndarray) -> np.ndarray:
    x = np.ascontiguousarray(np.asarray(x, dtype=np.float32))
    weight = np.ascontiguousarray(np.asarray(weight, dtype=np.float32))
    bias = np.ascontiguousarray(np.asarray(bias, dtype=np.float32))
    assert x.shape == (N_TOK, D_IN) and weight.shape == (D_OUT, D_IN)

    # (w > 0) - 0.5 equals 0.5*sign(w) only when no exact zeros exist;
    # fall back to the exact 3-op sign variant otherwise (bias zeros are
    # handled exactly by the Act-engine Sign either way).
    exact_sign = bool((weight == 0.0).any())
    nc = _get_nc(exact_sign)

    in_maps = []
    for tg in range(TOK_WAYS):
        for og in range(OUT_WAYS):
            in_maps.append({
                "x": np.ascontiguousarray(x[tg * TOK_SH:(tg + 1) * TOK_SH, :]),
                "w": np.ascontiguousarray(weight[og * OUT_SH:(og + 1) * OUT_SH, :]),
                "b": np.ascontiguousarray(
                    bias[og * OUT_SH:(og + 1) * OUT_SH].reshape(1, OUT_SH)),
            })

    res = run_bass_kernel_spmd(nc, in_maps, list(range(N_CORES)))

    out = np.empty((N_TOK, D_OUT), dtype=np.float32)
    c = 0
    for tg in range(TOK_WAYS):
        for og in range(OUT_WAYS):
            out[tg * TOK_SH:(tg + 1) * TOK_SH, og * OUT_SH:(og + 1) * OUT_SH] = \
                res.results[c]["out"]
            c += 1
    return out
